# revision 10
# baseline (speedup 1.0000x reference)
"""Trainium2 Bass kernel for nn_DirectMultiStepModel (2-layer graph-GRU + big Linear + softmax).

Self-contained: takes FULL inputs, shards nodes across 8 NeuronCores internally,
runs a single SPMD NEFF with on-device collectives, returns the FULL (1, 100) output.

Strategy:
  - Host: materialize the normalized adjacency as a dense padded matrix M
    (N=10000 -> 10240), shard destination nodes across cores (1280 each),
    pre-tiled fp8 for the aggregation matmuls. Fold the final Linear through
    the layer-2 aggregation on the host (G = lin_W . M), so the device skips
    aggregation 2 entirely and contracts G directly with h2 (feature-major).
  - Device per core: GRU1 (feature-major, h-part fp8 DoubleRow) -> transpose
    -> per-timestep AllGather of h1 (fp8) -> dense aggregation matmul (M^T
    resident in SBUF fp8, DoubleRow) with fused bias+ReLU into fp8 agg1 ->
    GRU2 (x-part fp8 DoubleRow) interleaved 2 steps behind GRU1 -> column-
    sharded matvec of h2 against G (fp8) -> AllReduce partial logits ->
    softmax.
"""
import sys
import types
import numpy as np
import ml_dtypes

import concourse.bass as bass
import concourse.bacc as bacc
import concourse.mybir as mybir
import concourse.tile as tile
from concourse.bass_utils import run_bass_kernel_spmd

BF16 = ml_dtypes.bfloat16
E4M3 = ml_dtypes.float8_e4m3
F32 = mybir.dt.float32
BF = mybir.dt.bfloat16
F8 = mybir.dt.float8e4
P = 128
GSCALE = 512.0  # host multiplies G by this; device divides at logit eviction


def _install_ntff_hook():
    """Register the NTFF profile hook the agent image's antenv lacks (no-op if present)."""
    try:
        import antenv.axon_hooks  # noqa: F401
        return
    except ImportError:
        pass
    try:
        import trn_agent_boot.trn_boot as tb
        hooks = types.ModuleType("antenv.axon_hooks")
        _h = [None]
        hooks.set_axon_ntff_profile_hook = lambda h: _h.__setitem__(0, h)
        hooks.get_axon_ntff_profile_hook = lambda: _h[0]
        sys.modules["antenv.axon_hooks"] = hooks
        import antenv
        antenv.axon_hooks = hooks
        hook = tb._ntff_profile_via_ctypes('/opt/axon/libaxon_pjrt.so')
        if hook is not None:
            hooks.set_axon_ntff_profile_hook(hook)
    except Exception:
        pass


class Cfg:
    def __init__(self, T=24, N=10000, DIN=128, H1=256, H2=128, OUT=100, NC=8, LAG=4):
        self.T, self.N, self.DIN, self.H1, self.H2, self.OUT, self.NC = T, N, DIN, H1, H2, OUT, NC
        self.NOWN = -(-N // (NC * P)) * P          # per-core padded node count
        self.NPAD = self.NOWN * NC                 # total padded nodes
        self.NT = self.NOWN // P                   # own node tiles
        self.CT = self.NPAD // P                   # contraction tiles
        self.PS1 = H1 // P                         # h1 feature partition-tiles
        self.PS2 = H2 // P
        self.G1, self.G2 = 3 * H1, 3 * H2
        self.LAG = LAG                             # agg/GRU2 runs LAG steps behind GRU1


# packed bias column offsets in the [P, 14] bias tile
B1RZ, B1HN, B1IN = 0, 4, 6
B2RZ, B2HN, B2IN = 8, 10, 11
CB1 = 12
NBIAS = 14


def fchunks(total, maxf=512):
    out, off = [], 0
    while off < total:
        fl = min(maxf, total - off)
        out.append((off, fl))
        off += fl
    return out


def build(cfg: Cfg):
    """Build + compile the SPMD kernel. Returns the compiled Bacc."""
    c = cfg
    nc = bacc.Bacc("TRN2", target_bir_lowering=False, debug=False, num_devices=c.NC)

    # ---- kernel I/O ----
    xT = nc.dram_tensor("xT", [c.T, c.DIN, c.NOWN], BF, kind="ExternalInput").ap()
    wih1T = nc.dram_tensor("wih1T", [c.DIN, c.G1], BF, kind="ExternalInput").ap()
    whh1d = nc.dram_tensor("whh1d", [P, 2 * c.G1], F8, kind="ExternalInput").ap()
    wih2d = nc.dram_tensor("wih2d", [P, 2 * c.G2], F8, kind="ExternalInput").ap()
    whh2T = nc.dram_tensor("whh2T", [c.H2, c.G2], BF, kind="ExternalInput").ap()
    biases = nc.dram_tensor("biases", [P, NBIAS], F32, kind="ExternalInput").ap()
    mTt = nc.dram_tensor("mTt", [4, P, c.CT, c.NOWN // 4], F8, kind="ExternalInput").ap()
    linW4 = nc.dram_tensor("linW4", [c.NT, c.H2, P, c.OUT], F8, kind="ExternalInput").ap()
    linc = nc.dram_tensor("linc", [1, c.OUT], F32, kind="ExternalInput").ap()
    ident = nc.dram_tensor("ident", [P, P], BF, kind="ExternalInput").ap()
    out = nc.dram_tensor("out", [1, c.OUT], F32, kind="ExternalOutput").ap()

    rg = [list(range(c.NC))]
    AGR = c.PS1 * P   # payload rows per core per timestep (node-major blocks)

    Sig = mybir.ActivationFunctionType.Sigmoid
    Tanh = mybir.ActivationFunctionType.Tanh
    Relu = mybir.ActivationFunctionType.Relu
    Exp = mybir.ActivationFunctionType.Exp
    Copy = mybir.ActivationFunctionType.Copy
    DR = mybir.MatmulPerfMode.DoubleRow

    with tile.TileContext(nc) as tc:
        with tc.tile_pool(name="dram", bufs=1, space="DRAM") as dram:
            ag_in = dram.tile([c.T, AGR, c.NOWN], F8)
            ag_outs = [dram.tile([AGR * c.NC, c.NOWN], F8, addr_space="Shared",
                                 name=f"ag_out{i}") for i in range(c.T)]
            ar_in = dram.tile([1, c.OUT], F32)
            ar_out = dram.tile([1, c.OUT], F32, addr_space="Shared")
            bar_in = dram.tile([1, P], F8)
            bar_out = dram.tile([c.NC, P], F8, addr_space="Shared")

            # ---- constants in SBUF (live for the whole kernel) ----
            with tc.tile_pool(name="const", bufs=1) as cpool:
                # rank barrier: absorb SPMD launch skew so the first real
                # AllGather isn't the one paying for it
                nc.gpsimd.collective_compute(
                    "AllGather", mybir.AluOpType.bypass, replica_groups=rg,
                    ins=[bar_in.opt()], outs=[bar_out.opt()])
                # critical-path loads first (GRU1 + transposes)
                wih1_sb = cpool.tile([P, c.G1], BF)
                nc.sync.dma_start(wih1_sb[:], wih1T[:])
                whh1_sb = cpool.tile([P, 2 * c.G1], F8)
                nc.sync.dma_start(whh1_sb[:], whh1d[:])
                bias_sb = cpool.tile([P, NBIAS], F32)
                nc.sync.dma_start(bias_sb[:], biases[:])
                ident_sb = cpool.tile([P, P], BF)
                nc.sync.dma_start(ident_sb[:], ident[:])
                wih2_sb = cpool.tile([P, 2 * c.G2], F8)
                nc.sync.dma_start(wih2_sb[:], wih2d[:])
                whh2_sb = cpool.tile([P, c.G2], BF)
                nc.sync.dma_start(whh2_sb[:], whh2T[:])
                linc_sb = cpool.tile([1, c.OUT], F32)
                nc.sync.dma_start(linc_sb[:], linc[:])

                whh1_3 = whh1_sb[:].rearrange("p (c g) -> p c g", g=c.G1)
                wih2_3 = wih2_sb[:].rearrange("p (c g) -> p c g", g=c.G2)

                # h states (live through the tail)
                with tc.tile_pool(name="hstate", bufs=1) as hpool:
                    h1 = hpool.tile([P, c.PS1 * c.NOWN], BF)
                    h1f8 = hpool.tile([P, c.PS1 * c.NOWN], F8)
                    h2 = hpool.tile([P, c.NOWN], BF)
                    nc.vector.memset(h1[:], 0.0)
                    nc.vector.memset(h1f8[:], 0.0)
                    nc.vector.memset(h2[:], 0.0)
                    h1f8_3 = h1f8[:].rearrange("p (c n) -> p c n", n=c.NOWN)

                    def gru1_step(xt, work, psum, nofs, nlen):
                        """GRU1 over nodes [nofs, nofs+nlen); h-part fp8 DoubleRow."""
                        NOWN = c.NOWN
                        FCg = fchunks(nlen)
                        rz = work.tile([P, 2 * c.PS1 * nlen], BF, tag="rz")
                        nsb = work.tile([P, c.PS1 * nlen], BF, tag="nsb")
                        insb = work.tile([P, c.PS1 * nlen], BF, tag="insb")
                        hnsb = work.tile([P, c.PS1 * nlen], BF, tag="hnsb")
                        for g in range(2 * c.PS1):
                            for (fo, fl) in FCg:
                                no = nofs + fo
                                pt = psum.tile([P, fl], F32, tag="psg")
                                nc.tensor.matmul(pt[:], wih1_sb[:, g * P:(g + 1) * P],
                                                 xt[:, no:no + fl], start=True, stop=False)
                                nc.tensor.matmul(pt[:], whh1_3[:, :, g * P:(g + 1) * P],
                                                 h1f8_3[:, :, no:no + fl],
                                                 start=False, stop=True, perf_mode=DR)
                                nc.scalar.activation(rz[:, g * nlen + fo:g * nlen + fo + fl],
                                                     pt[:], Sig,
                                                     bias=bias_sb[:, B1RZ + g:B1RZ + g + 1])
                        for g2 in range(c.PS1):
                            gofs = (2 * c.PS1 + g2) * P
                            for (fo, fl) in FCg:
                                no = nofs + fo
                                pi = psum.tile([P, fl], F32, tag="psg")
                                nc.tensor.matmul(pi[:], wih1_sb[:, gofs:gofs + P],
                                                 xt[:, no:no + fl], start=True, stop=True)
                                nc.vector.tensor_scalar_add(
                                    insb[:, g2 * nlen + fo:g2 * nlen + fo + fl], pi[:],
                                    bias_sb[:, B1IN + g2:B1IN + g2 + 1])
                                ph = psum.tile([P, fl], F32, tag="psg")
                                nc.tensor.matmul(ph[:], whh1_3[:, :, gofs:gofs + P],
                                                 h1f8_3[:, :, no:no + fl],
                                                 start=True, stop=True, perf_mode=DR)
                                nc.vector.tensor_scalar_add(
                                    hnsb[:, g2 * nlen + fo:g2 * nlen + fo + fl], ph[:],
                                    bias_sb[:, B1HN + g2:B1HN + g2 + 1])
                        for g2 in range(c.PS1):
                            sl = slice(g2 * nlen, (g2 + 1) * nlen)
                            z_sl = slice((c.PS1 + g2) * nlen, (c.PS1 + g2 + 1) * nlen)
                            h_sl = slice(g2 * NOWN + nofs, g2 * NOWN + nofs + nlen)
                            nc.vector.tensor_mul(hnsb[:, sl], rz[:, sl], hnsb[:, sl])
                            nc.vector.tensor_add(hnsb[:, sl], hnsb[:, sl], insb[:, sl])
                            nc.scalar.activation(nsb[:, sl], hnsb[:, sl], Tanh)
                            nc.vector.tensor_sub(hnsb[:, sl], h1[:, h_sl], nsb[:, sl])
                            nc.vector.tensor_mul(hnsb[:, sl], rz[:, z_sl], hnsb[:, sl])
                            nc.vector.tensor_add(h1[:, h_sl], nsb[:, sl], hnsb[:, sl])
                            nc.vector.tensor_copy(h1f8[:, h_sl], h1[:, h_sl])

                    def gru2_step(a1t_3, work, psum, nofs, nlen):
                        """GRU2 over nodes [nofs, nofs+nlen); x-part fp8 DoubleRow."""
                        FCg = fchunks(nlen)
                        rz = work.tile([P, 2 * nlen], BF, tag="rz2")
                        nsb = work.tile([P, nlen], BF, tag="nsb2")
                        insb = work.tile([P, nlen], BF, tag="insb2")
                        hnsb = work.tile([P, nlen], BF, tag="hnsb2")
                        for g in range(2):
                            for (fo, fl) in FCg:
                                no = nofs + fo
                                pt = psum.tile([P, fl], F32, tag="psg")
                                nc.tensor.matmul(pt[:], wih2_3[:, :, g * P:(g + 1) * P],
                                                 a1t_3[:, :, no:no + fl],
                                                 start=True, stop=False, perf_mode=DR)
                                nc.tensor.matmul(pt[:], whh2_sb[:, g * P:(g + 1) * P],
                                                 h2[:, no:no + fl], start=False, stop=True)
                                nc.scalar.activation(rz[:, g * nlen + fo:g * nlen + fo + fl],
                                                     pt[:], Sig,
                                                     bias=bias_sb[:, B2RZ + g:B2RZ + g + 1])
                        gofs = 2 * P
                        for (fo, fl) in FCg:
                            no = nofs + fo
                            pi = psum.tile([P, fl], F32, tag="psg")
                            nc.tensor.matmul(pi[:], wih2_3[:, :, gofs:gofs + P],
                                             a1t_3[:, :, no:no + fl],
                                             start=True, stop=True, perf_mode=DR)
                            nc.vector.tensor_scalar_add(insb[:, fo:fo + fl], pi[:],
                                                        bias_sb[:, B2IN:B2IN + 1])
                            ph = psum.tile([P, fl], F32, tag="psg")
                            nc.tensor.matmul(ph[:], whh2_sb[:, gofs:gofs + P],
                                             h2[:, no:no + fl], start=True, stop=True)
                            nc.vector.tensor_scalar_add(hnsb[:, fo:fo + fl], ph[:],
                                                        bias_sb[:, B2HN:B2HN + 1])
                        sl = slice(0, nlen)
                        z_sl = slice(nlen, 2 * nlen)
                        h_sl = slice(nofs, nofs + nlen)
                        nc.vector.tensor_mul(hnsb[:, sl], rz[:, sl], hnsb[:, sl])
                        nc.vector.tensor_add(hnsb[:, sl], hnsb[:, sl], insb[:, sl])
                        nc.scalar.activation(nsb[:, sl], hnsb[:, sl], Tanh)
                        nc.vector.tensor_sub(hnsb[:, sl], h2[:, h_sl], nsb[:, sl])
                        nc.vector.tensor_mul(hnsb[:, sl], rz[:, z_sl], hnsb[:, sl])
                        nc.vector.tensor_add(h2[:, h_sl], nsb[:, sl], hnsb[:, sl])

                    # ===== main pipeline =====
                    with tc.tile_pool(name="mtp", bufs=1) as mtp:
                        QT = c.NOWN // 4
                        GW = c.NOWN // 2
                        mtq = []
                        for q in range(4):
                            mq = mtp.tile([P, c.CT * QT], F8, name=f"mtq{q}")
                            half = c.CT * QT // 2
                            # off the sync queue so xT/weights aren't stuck behind 13MB
                            nc.scalar.dma_start(mq[:, :half], mTt[q, :, :c.CT // 2, :])
                            nc.scalar.dma_start(mq[:, half:], mTt[q, :, c.CT // 2:, :])
                            mtq.append(mq)
                        with tc.tile_pool(name="p1x", bufs=3) as p1x, \
                             tc.tile_pool(name="p1w", bufs=2) as p1w, \
                             tc.tile_pool(name="psg", bufs=3, space="PSUM") as psg, \
                             tc.tile_pool(name="ps1t", bufs=2, space="PSUM") as ps1t, \
                             tc.tile_pool(name="p1s", bufs=2) as p1s, \
                             tc.tile_pool(name="hstp", bufs=2) as hstp, \
                             tc.tile_pool(name="a1p", bufs=2) as a1p, \
                             tc.tile_pool(name="p3w", bufs=2) as p3w, \
                             tc.tile_pool(name="aps", bufs=2, space="PSUM") as aps:
                            xts = {}
                            for step in range(c.T + c.LAG):
                                # ---- gathered-h loads for agg u, first on the sync queue ----
                                if step >= c.LAG:
                                    u = step - c.LAG
                                    hsts = []
                                    for ps in range(c.PS1):
                                        hst = hstp.tile([P, c.CT * P], F8, tag="hst")
                                        for r in range(c.NC):
                                            ro = (r * c.PS1 + ps) * P
                                            nc.sync.dma_start(
                                                hst[:, r * c.NOWN:(r + 1) * c.NOWN],
                                                ag_outs[u][ro:ro + P, :])
                                        hsts.append(hst)
                                if step < c.T:
                                    t = step
                                    if t == 0:
                                        xts[0] = p1x.tile([P, c.NOWN], BF, tag="xt", name="xt0")
                                        nc.sync.dma_start(xts[0][:], xT[0])
                                    if t + 1 < c.T:
                                        xts[t + 1] = p1x.tile([P, c.NOWN], BF, tag="xt", name=f"xt{t+1}")
                                        nc.sync.dma_start(xts[t + 1][:], xT[t + 1])
                                    xt = xts.pop(t)
                                    for grp in range(2):
                                        gru1_step(xt, p1w, psg, grp * GW, GW)
                                    # transpose h1_t to node-major (fp8), stage + AllGather
                                    for ps in range(c.PS1):
                                        stg = p1s.tile([P, c.NOWN], F8, tag="stg")
                                        for nt in range(c.NT):
                                            pt = ps1t.tile([P, P], BF, tag="ps_t")
                                            nc.tensor.transpose(
                                                pt[:],
                                                h1[:, ps * c.NOWN + nt * P:ps * c.NOWN + (nt + 1) * P],
                                                ident_sb[:])
                                            nc.vector.tensor_copy(stg[:, nt * P:(nt + 1) * P], pt[:])
                                        nc.sync.dma_start(ag_in[t, ps * P:(ps + 1) * P, :], stg[:])
                                    nc.gpsimd.collective_compute(
                                        "AllGather", mybir.AluOpType.bypass, replica_groups=rg,
                                        ins=[ag_in[t].opt()], outs=[ag_outs[t].opt()])
                                if step >= c.LAG:
                                    u = step - c.LAG
                                    a1t = a1p.tile([P, c.PS1 * c.NOWN], F8, tag="a1t")
                                    for q in range(4):
                                        mt3 = mtq[q][:].rearrange("p (ct f) -> p ct f", f=QT)
                                        for ps in range(c.PS1):
                                            hst3 = hsts[ps][:].rearrange("p (ct f) -> p ct f", f=P)
                                            pa = aps.tile([P, QT], F32, tag="pa")
                                            for cp in range(c.CT // 2):
                                                nc.tensor.matmul(
                                                    pa[:], hst3[:, 2 * cp:2 * cp + 2, :],
                                                    mt3[:, 2 * cp:2 * cp + 2, :],
                                                    start=(cp == 0), stop=(cp == c.CT // 2 - 1),
                                                    perf_mode=DR)
                                            nc.scalar.activation(
                                                a1t[:, ps * c.NOWN + q * QT:ps * c.NOWN + (q + 1) * QT],
                                                pa[:], Relu, bias=bias_sb[:, CB1 + ps:CB1 + ps + 1])
                                    a1t_3 = a1t[:].rearrange("p (c n) -> p c n", n=c.NOWN)
                                    for grp in range(2):
                                        gru2_step(a1t_3, p3w, psg, grp * GW, GW)

                    # ---- tail: h2 . G matvec + AllReduce + softmax ----
                    HC = 64
                    NCK = c.NOWN // HC
                    with tc.tile_pool(name="p4", bufs=1) as p4, \
                         tc.tile_pool(name="p4w", bufs=NCK) as p4w, \
                         tc.tile_pool(name="ps4", bufs=1, space="PSUM") as ps4:
                        plog = ps4.tile([1, c.OUT], F32, tag="plog")
                        # prefetch all of G during the pipeline drain
                        lws = []
                        for ck in range(NCK):
                            n0 = ck * HC
                            nt, no = n0 // P, n0 % P
                            lw = p4w.tile([P, HC * c.OUT], F8, tag="lw", name=f"lw{ck}")
                            nc.sync.dma_start(
                                lw[:].rearrange("p (n j) -> p n j", j=c.OUT),
                                linW4[nt, :, no:no + HC, :])
                            lws.append(lw)
                        n_mm = 0
                        for ck in range(NCK):
                            n0 = ck * HC
                            lw = lws[ck]
                            for ni in range(HC):
                                nc.tensor.matmul(plog[:], h2[:, n0 + ni:n0 + ni + 1],
                                                 lw[:, ni * c.OUT:(ni + 1) * c.OUT],
                                                 start=(n_mm == 0),
                                                 stop=(n_mm == c.NOWN - 1))
                                n_mm += 1
                        lpart = p4.tile([1, c.OUT], F32)
                        nc.scalar.activation(lpart[:], plog[:], Copy, scale=1.0 / GSCALE)
                        nc.sync.dma_start(ar_in[:], lpart[:])
                        nc.gpsimd.collective_compute(
                            "AllReduce", mybir.AluOpType.add, replica_groups=rg,
                            ins=[ar_in.opt()], outs=[ar_out.opt()])
                        lg = p4.tile([1, c.OUT], F32)
                        nc.sync.dma_start(lg[:], ar_out[:])
                        nc.vector.tensor_add(lg[:], lg[:], linc_sb[:])
                        mx = p4.tile([1, 1], F32)
                        nc.vector.tensor_reduce(mx[:], lg[:], mybir.AxisListType.X,
                                                mybir.AluOpType.max, negate=True)
                        ex = p4.tile([1, c.OUT], F32)
                        nc.scalar.activation(ex[:], lg[:], Exp, bias=mx[:, 0:1])
                        sm = p4.tile([1, 1], F32)
                        nc.vector.tensor_reduce(sm[:], ex[:], mybir.AxisListType.X,
                                                mybir.AluOpType.add)
                        rcp = p4.tile([1, 1], F32)
                        nc.vector.reciprocal(rcp[:], sm[:])
                        res = p4.tile([1, c.OUT], F32)
                        nc.vector.tensor_scalar_mul(res[:], ex[:], rcp[:, 0:1])
                        nc.sync.dma_start(out[:], res[:])

    nc.compile()
    return nc


def host_prep(cfg: Cfg, x, edge_index, W_ih1, W_hh1, b_ih1, b_hh1, bias1,
              W_ih2, W_hh2, b_ih2, b_hh2, bias2, lin_W, lin_b):
    """Shard + preprocess FULL inputs into per-core in_maps."""
    import scipy.sparse as sp
    c = cfg
    x = np.asarray(x, np.float32)
    edge_index = np.asarray(edge_index)
    # normalized adjacency with self loops: M[dst, src]
    row, col = edge_index[0], edge_index[1]
    loops = np.arange(c.N, dtype=row.dtype)
    row = np.concatenate([row, loops])
    col = np.concatenate([col, loops])
    deg = np.zeros(c.N, np.float32)
    np.add.at(deg, col, 1.0)
    dis = np.where(deg > 0, deg ** -0.5, 0.0).astype(np.float32)
    norm = (dis[row] * dis[col]).astype(np.float32)
    M = np.zeros((c.NPAD, c.NPAD), np.float32)
    np.add.at(M, (col, row), norm)

    xp = np.zeros((c.T, c.NPAD, c.DIN), np.float32)
    xp[:, :c.N, :] = x

    W_ih1 = np.asarray(W_ih1, np.float32); W_hh1 = np.asarray(W_hh1, np.float32)
    W_ih2 = np.asarray(W_ih2, np.float32); W_hh2 = np.asarray(W_hh2, np.float32)
    b_ih1 = np.asarray(b_ih1, np.float32); b_hh1 = np.asarray(b_hh1, np.float32)
    b_ih2 = np.asarray(b_ih2, np.float32); b_hh2 = np.asarray(b_hh2, np.float32)
    bias1 = np.asarray(bias1, np.float32); bias2 = np.asarray(bias2, np.float32)
    lin_W = np.asarray(lin_W, np.float32); lin_b = np.asarray(lin_b, np.float32)

    def dr_layout(WT, G):
        # [2*128, G] -> [128, 2, G] -> [128, 2*G] fp8 (DoubleRow stationary)
        return np.ascontiguousarray(
            WT.reshape(2, P, G).transpose(1, 0, 2).reshape(P, 2 * G)).astype(E4M3)

    bias_cols = np.zeros((P, NBIAS), np.float32)
    bias_cols[:, B1RZ:B1RZ + 4] = (b_ih1 + b_hh1)[:2 * c.H1].reshape(4, P).T
    bias_cols[:, B1HN:B1HN + 2] = b_hh1[2 * c.H1:].reshape(2, P).T
    bias_cols[:, B1IN:B1IN + 2] = b_ih1[2 * c.H1:].reshape(2, P).T
    bias_cols[:, B2RZ:B2RZ + 2] = (b_ih2 + b_hh2)[:2 * c.H2].reshape(2, P).T
    bias_cols[:, B2HN] = b_hh2[2 * c.H2:]
    bias_cols[:, B2IN] = b_ih2[2 * c.H2:]
    bias_cols[:, CB1:CB1 + 2] = bias1.reshape(2, P).T

    # fold the final Linear through aggregation 2:
    #   logits = sum_{s,f} G[o,s,f] h2[s,f] + linc,  G = einsum('odf,ds->osf')
    lw = np.zeros((c.OUT, c.NPAD, c.H2), np.float32)
    lw[:, :c.N, :] = lin_W.reshape(c.OUT, c.N, c.H2)
    St = sp.coo_matrix((norm, (row, col)), shape=(c.NPAD, c.NPAD)).tocsr()  # M^T
    LWd = np.ascontiguousarray(lw.transpose(1, 0, 2).reshape(c.NPAD, c.OUT * c.H2))
    G = St @ LWd  # (NPAD, OUT*H2)
    G = np.clip(G * GSCALE, -240.0, 240.0)
    linc_v = (np.einsum('odf,f->o', lw, bias2) + lin_b).reshape(1, c.OUT).astype(np.float32)

    common = dict(
        wih1T=W_ih1.T.astype(BF16),
        whh1d=dr_layout(W_hh1.T.astype(np.float32), c.G1),
        wih2d=dr_layout(W_ih2.T.astype(np.float32), c.G2),
        whh2T=W_hh2.T.astype(BF16),
        biases=bias_cols,
        linc=linc_v,
        ident=np.eye(P, dtype=BF16),
    )

    in_maps = []
    for k in range(c.NC):
        sl = slice(k * c.NOWN, (k + 1) * c.NOWN)
        m = dict(common)
        m["xT"] = np.ascontiguousarray(xp[:, sl, :].transpose(0, 2, 1)).astype(BF16)
        # M^T slice for this core's dest nodes, pre-tiled: (4, P, CT, NOWN//4)
        mk = M[sl, :].T.reshape(c.CT, P, 4, c.NOWN // 4)
        m["mTt"] = np.ascontiguousarray(mk.transpose(2, 1, 0, 3)).astype(E4M3)
        # G slice: (NOWN, OUT*H2) -> (NT, H2, P, OUT)
        gk = G[sl].reshape(c.NT, P, c.OUT, c.H2)
        m["linW4"] = np.ascontiguousarray(gk.transpose(0, 3, 1, 2)).astype(E4M3)
        in_maps.append(m)
    return in_maps


_CACHE = {}


def _get_built(key, cfg):
    if key not in _CACHE:
        _CACHE[key] = build(cfg)
    return _CACHE[key]


def run(cfg: Cfg, inputs, trace=False):
    _install_ntff_hook()
    nc = _get_built(("cfg", cfg.T, cfg.N), cfg)
    in_maps = host_prep(cfg, **inputs)
    res = run_bass_kernel_spmd(nc, in_maps, core_ids=list(range(cfg.NC)), trace=trace)
    return res


def kernel(**inputs) -> np.ndarray:
    cfg = Cfg()
    res = run(cfg, inputs)
    return np.asarray(res.results[0]["out"], np.float32)


# revision 12
# speedup vs baseline: 1.0995x; 1.0995x over previous
"""Trainium2 Bass kernel for nn_DirectMultiStepModel (2-layer graph-GRU + big Linear + softmax).

Self-contained: takes FULL inputs, shards nodes across 8 NeuronCores internally,
runs a single SPMD NEFF with on-device collectives, returns the FULL (1, 100) output.

Strategy:
  - Host: materialize the normalized adjacency as a dense padded matrix M
    (N=10000 -> 10240), shard destination nodes across cores (1280 each),
    pre-tiled fp8 for the aggregation matmuls. Fold the final Linear through
    the layer-2 aggregation on the host (G = lin_W . M), so the device skips
    aggregation 2 entirely and contracts G directly with h2 (feature-major).
  - Device per core: GRU1 (feature-major, h-part fp8 DoubleRow) -> transpose
    -> per-timestep AllGather of h1 (fp8) -> dense aggregation matmul (M^T
    resident in SBUF fp8, DoubleRow) with fused bias+ReLU into fp8 agg1 ->
    GRU2 (x-part fp8 DoubleRow) interleaved 2 steps behind GRU1 -> column-
    sharded matvec of h2 against G (fp8) -> AllReduce partial logits ->
    softmax.
"""
import sys
import types
import numpy as np
import ml_dtypes

import concourse.bass as bass
import concourse.bacc as bacc
import concourse.mybir as mybir
import concourse.tile as tile
from concourse.bass_utils import run_bass_kernel_spmd

BF16 = ml_dtypes.bfloat16
E4M3 = ml_dtypes.float8_e4m3
F32 = mybir.dt.float32
BF = mybir.dt.bfloat16
F8 = mybir.dt.float8e4
P = 128
GSCALE = 512.0  # host multiplies G by this; device divides at logit eviction


def _install_ntff_hook():
    """Register the NTFF profile hook the agent image's antenv lacks (no-op if present)."""
    try:
        import antenv.axon_hooks  # noqa: F401
        return
    except ImportError:
        pass
    try:
        import trn_agent_boot.trn_boot as tb
        hooks = types.ModuleType("antenv.axon_hooks")
        _h = [None]
        hooks.set_axon_ntff_profile_hook = lambda h: _h.__setitem__(0, h)
        hooks.get_axon_ntff_profile_hook = lambda: _h[0]
        sys.modules["antenv.axon_hooks"] = hooks
        import antenv
        antenv.axon_hooks = hooks
        hook = tb._ntff_profile_via_ctypes('/opt/axon/libaxon_pjrt.so')
        if hook is not None:
            hooks.set_axon_ntff_profile_hook(hook)
    except Exception:
        pass


class Cfg:
    def __init__(self, T=24, N=10000, DIN=128, H1=256, H2=128, OUT=100, NC=8, LAG=4):
        self.T, self.N, self.DIN, self.H1, self.H2, self.OUT, self.NC = T, N, DIN, H1, H2, OUT, NC
        self.NOWN = -(-N // (NC * P)) * P          # per-core padded node count
        self.NPAD = self.NOWN * NC                 # total padded nodes
        self.NT = self.NOWN // P                   # own node tiles
        self.CT = self.NPAD // P                   # contraction tiles
        self.PS1 = H1 // P                         # h1 feature partition-tiles
        self.PS2 = H2 // P
        self.G1, self.G2 = 3 * H1, 3 * H2
        self.LAG = LAG                             # agg/GRU2 runs LAG steps behind GRU1


# packed bias column offsets in the [P, 14] bias tile
B1RZ, B1HN, B1IN = 0, 4, 6
B2RZ, B2HN, B2IN = 8, 10, 11
CB1 = 12
NBIAS = 14


def fchunks(total, maxf=512):
    out, off = [], 0
    while off < total:
        fl = min(maxf, total - off)
        out.append((off, fl))
        off += fl
    return out


def build(cfg: Cfg):
    """Build + compile the SPMD kernel. Returns the compiled Bacc."""
    c = cfg
    nc = bacc.Bacc("TRN2", target_bir_lowering=False, debug=False, num_devices=c.NC)

    # ---- kernel I/O ----
    xT = nc.dram_tensor("xT", [c.T, c.DIN, c.NOWN], BF, kind="ExternalInput").ap()
    wih1T = nc.dram_tensor("wih1T", [c.DIN, c.G1], BF, kind="ExternalInput").ap()
    whh1d = nc.dram_tensor("whh1d", [P, 2 * c.G1], F8, kind="ExternalInput").ap()
    wih2d = nc.dram_tensor("wih2d", [P, 2 * c.G2], F8, kind="ExternalInput").ap()
    whh2T = nc.dram_tensor("whh2T", [c.H2, c.G2], BF, kind="ExternalInput").ap()
    biases = nc.dram_tensor("biases", [P, NBIAS], F32, kind="ExternalInput").ap()
    mTt = nc.dram_tensor("mTt", [4, P, c.CT, c.NOWN // 4], F8, kind="ExternalInput").ap()
    linW4 = nc.dram_tensor("linW4", [c.NT, c.H2, P, c.OUT], F8, kind="ExternalInput").ap()
    linc = nc.dram_tensor("linc", [1, c.OUT], F32, kind="ExternalInput").ap()
    ident = nc.dram_tensor("ident", [P, P], BF, kind="ExternalInput").ap()
    out = nc.dram_tensor("out", [1, c.OUT], F32, kind="ExternalOutput").ap()

    rg = [list(range(c.NC))]
    AGR = c.PS1 * P   # payload rows per core per timestep (node-major blocks)

    Sig = mybir.ActivationFunctionType.Sigmoid
    Tanh = mybir.ActivationFunctionType.Tanh
    Relu = mybir.ActivationFunctionType.Relu
    Exp = mybir.ActivationFunctionType.Exp
    Copy = mybir.ActivationFunctionType.Copy
    DR = mybir.MatmulPerfMode.DoubleRow

    with tile.TileContext(nc) as tc:
        with tc.tile_pool(name="dram", bufs=1, space="DRAM") as dram:
            ag_in = dram.tile([c.T, AGR, c.NOWN], F8)
            ag_outs = [dram.tile([AGR * c.NC, c.NOWN], F8, addr_space="Shared",
                                 name=f"ag_out{i}") for i in range(c.T)]
            ar_in = dram.tile([1, c.OUT], F32)
            ar_out = dram.tile([1, c.OUT], F32, addr_space="Shared")
            bar_in = dram.tile([1, P], F8)
            bar_out = dram.tile([c.NC, P], F8, addr_space="Shared")

            # ---- constants in SBUF (live for the whole kernel) ----
            with tc.tile_pool(name="const", bufs=1) as cpool:
                # rank barrier: absorb SPMD launch skew so the first real
                # AllGather isn't the one paying for it
                nc.gpsimd.collective_compute(
                    "AllGather", mybir.AluOpType.bypass, replica_groups=rg,
                    ins=[bar_in.opt()], outs=[bar_out.opt()])
                # critical-path loads first (GRU1 + transposes)
                wih1_sb = cpool.tile([P, c.G1], BF)
                nc.sync.dma_start(wih1_sb[:], wih1T[:])
                whh1_sb = cpool.tile([P, 2 * c.G1], F8)
                nc.sync.dma_start(whh1_sb[:], whh1d[:])
                bias_sb = cpool.tile([P, NBIAS], F32)
                nc.sync.dma_start(bias_sb[:], biases[:])
                ident_sb = cpool.tile([P, P], BF)
                nc.sync.dma_start(ident_sb[:], ident[:])
                wih2_sb = cpool.tile([P, 2 * c.G2], F8)
                nc.sync.dma_start(wih2_sb[:], wih2d[:])
                whh2_sb = cpool.tile([P, c.G2], BF)
                nc.sync.dma_start(whh2_sb[:], whh2T[:])
                linc_sb = cpool.tile([1, c.OUT], F32)
                nc.sync.dma_start(linc_sb[:], linc[:])

                whh1_3 = whh1_sb[:].rearrange("p (c g) -> p c g", g=c.G1)
                wih2_3 = wih2_sb[:].rearrange("p (c g) -> p c g", g=c.G2)

                # h states (live through the tail)
                with tc.tile_pool(name="hstate", bufs=1) as hpool:
                    h1 = hpool.tile([P, c.PS1 * c.NOWN], BF)
                    h1f8 = hpool.tile([P, c.PS1 * c.NOWN], F8)
                    h2 = hpool.tile([P, c.NOWN], BF)
                    nc.vector.memset(h1[:], 0.0)
                    nc.vector.memset(h1f8[:], 0.0)
                    nc.vector.memset(h2[:], 0.0)
                    h1f8_3 = h1f8[:].rearrange("p (c n) -> p c n", n=c.NOWN)

                    def gru1_step(xt, work, psum, nofs, nlen):
                        """GRU1 over nodes [nofs, nofs+nlen); h-part fp8 DoubleRow."""
                        NOWN = c.NOWN
                        FCg = fchunks(nlen)
                        rz = work.tile([P, 2 * c.PS1 * nlen], BF, tag="rz")
                        nsb = work.tile([P, c.PS1 * nlen], BF, tag="nsb")
                        insb = work.tile([P, c.PS1 * nlen], BF, tag="insb")
                        hnsb = work.tile([P, c.PS1 * nlen], BF, tag="hnsb")
                        for g in range(2 * c.PS1):
                            for (fo, fl) in FCg:
                                no = nofs + fo
                                pt = psum.tile([P, fl], F32, tag="psg")
                                nc.tensor.matmul(pt[:], wih1_sb[:, g * P:(g + 1) * P],
                                                 xt[:, no:no + fl], start=True, stop=False)
                                nc.tensor.matmul(pt[:], whh1_3[:, :, g * P:(g + 1) * P],
                                                 h1f8_3[:, :, no:no + fl],
                                                 start=False, stop=True, perf_mode=DR)
                                nc.scalar.activation(rz[:, g * nlen + fo:g * nlen + fo + fl],
                                                     pt[:], Sig,
                                                     bias=bias_sb[:, B1RZ + g:B1RZ + g + 1])
                        for g2 in range(c.PS1):
                            gofs = (2 * c.PS1 + g2) * P
                            for (fo, fl) in FCg:
                                no = nofs + fo
                                pi = psum.tile([P, fl], F32, tag="psg")
                                nc.tensor.matmul(pi[:], wih1_sb[:, gofs:gofs + P],
                                                 xt[:, no:no + fl], start=True, stop=True)
                                nc.vector.tensor_scalar_add(
                                    insb[:, g2 * nlen + fo:g2 * nlen + fo + fl], pi[:],
                                    bias_sb[:, B1IN + g2:B1IN + g2 + 1])
                                ph = psum.tile([P, fl], F32, tag="psg")
                                nc.tensor.matmul(ph[:], whh1_3[:, :, gofs:gofs + P],
                                                 h1f8_3[:, :, no:no + fl],
                                                 start=True, stop=True, perf_mode=DR)
                                nc.vector.tensor_scalar_add(
                                    hnsb[:, g2 * nlen + fo:g2 * nlen + fo + fl], ph[:],
                                    bias_sb[:, B1HN + g2:B1HN + g2 + 1])
                        for g2 in range(c.PS1):
                            sl = slice(g2 * nlen, (g2 + 1) * nlen)
                            z_sl = slice((c.PS1 + g2) * nlen, (c.PS1 + g2 + 1) * nlen)
                            h_sl = slice(g2 * NOWN + nofs, g2 * NOWN + nofs + nlen)
                            nc.vector.tensor_mul(hnsb[:, sl], rz[:, sl], hnsb[:, sl])
                            nc.vector.tensor_add(hnsb[:, sl], hnsb[:, sl], insb[:, sl])
                            nc.scalar.activation(nsb[:, sl], hnsb[:, sl], Tanh)
                            nc.gpsimd.tensor_sub(hnsb[:, sl], h1[:, h_sl], nsb[:, sl])
                            nc.vector.tensor_mul(hnsb[:, sl], rz[:, z_sl], hnsb[:, sl])
                            nc.gpsimd.tensor_add(h1[:, h_sl], nsb[:, sl], hnsb[:, sl])
                            nc.vector.tensor_copy(h1f8[:, h_sl], h1[:, h_sl])

                    def gru2_step(a1t_3, work, psum, nofs, nlen):
                        """GRU2 over nodes [nofs, nofs+nlen); x-part fp8 DoubleRow."""
                        FCg = fchunks(nlen)
                        rz = work.tile([P, 2 * nlen], BF, tag="rz2")
                        nsb = work.tile([P, nlen], BF, tag="nsb2")
                        insb = work.tile([P, nlen], BF, tag="insb2")
                        hnsb = work.tile([P, nlen], BF, tag="hnsb2")
                        for g in range(2):
                            for (fo, fl) in FCg:
                                no = nofs + fo
                                pt = psum.tile([P, fl], F32, tag="psg")
                                nc.tensor.matmul(pt[:], wih2_3[:, :, g * P:(g + 1) * P],
                                                 a1t_3[:, :, no:no + fl],
                                                 start=True, stop=False, perf_mode=DR)
                                nc.tensor.matmul(pt[:], whh2_sb[:, g * P:(g + 1) * P],
                                                 h2[:, no:no + fl], start=False, stop=True)
                                nc.scalar.activation(rz[:, g * nlen + fo:g * nlen + fo + fl],
                                                     pt[:], Sig,
                                                     bias=bias_sb[:, B2RZ + g:B2RZ + g + 1])
                        gofs = 2 * P
                        for (fo, fl) in FCg:
                            no = nofs + fo
                            pi = psum.tile([P, fl], F32, tag="psg")
                            nc.tensor.matmul(pi[:], wih2_3[:, :, gofs:gofs + P],
                                             a1t_3[:, :, no:no + fl],
                                             start=True, stop=True, perf_mode=DR)
                            nc.vector.tensor_scalar_add(insb[:, fo:fo + fl], pi[:],
                                                        bias_sb[:, B2IN:B2IN + 1])
                            ph = psum.tile([P, fl], F32, tag="psg")
                            nc.tensor.matmul(ph[:], whh2_sb[:, gofs:gofs + P],
                                             h2[:, no:no + fl], start=True, stop=True)
                            nc.vector.tensor_scalar_add(hnsb[:, fo:fo + fl], ph[:],
                                                        bias_sb[:, B2HN:B2HN + 1])
                        sl = slice(0, nlen)
                        z_sl = slice(nlen, 2 * nlen)
                        h_sl = slice(nofs, nofs + nlen)
                        nc.vector.tensor_mul(hnsb[:, sl], rz[:, sl], hnsb[:, sl])
                        nc.vector.tensor_add(hnsb[:, sl], hnsb[:, sl], insb[:, sl])
                        nc.scalar.activation(nsb[:, sl], hnsb[:, sl], Tanh)
                        nc.gpsimd.tensor_sub(hnsb[:, sl], h2[:, h_sl], nsb[:, sl])
                        nc.vector.tensor_mul(hnsb[:, sl], rz[:, z_sl], hnsb[:, sl])
                        nc.gpsimd.tensor_add(h2[:, h_sl], nsb[:, sl], hnsb[:, sl])

                    # ===== main pipeline =====
                    with tc.tile_pool(name="mtp", bufs=1) as mtp:
                        QT = c.NOWN // 4
                        GW = c.NOWN // 2
                        mtq = []
                        for q in range(4):
                            mq = mtp.tile([P, c.CT * QT], F8, name=f"mtq{q}")
                            half = c.CT * QT // 2
                            # off the sync queue so xT/weights aren't stuck behind 13MB
                            nc.scalar.dma_start(mq[:, :half], mTt[q, :, :c.CT // 2, :])
                            nc.scalar.dma_start(mq[:, half:], mTt[q, :, c.CT // 2:, :])
                            mtq.append(mq)
                        with tc.tile_pool(name="p1x", bufs=3) as p1x, \
                             tc.tile_pool(name="p1w", bufs=2) as p1w, \
                             tc.tile_pool(name="psg", bufs=3, space="PSUM") as psg, \
                             tc.tile_pool(name="ps1t", bufs=2, space="PSUM") as ps1t, \
                             tc.tile_pool(name="p1s", bufs=2) as p1s, \
                             tc.tile_pool(name="hstp", bufs=2) as hstp, \
                             tc.tile_pool(name="a1p", bufs=2) as a1p, \
                             tc.tile_pool(name="p3w", bufs=2) as p3w, \
                             tc.tile_pool(name="aps", bufs=2, space="PSUM") as aps:
                            xts = {}
                            for step in range(c.T + c.LAG):
                                # ---- gathered-h loads for agg u, first on the sync queue ----
                                if step >= c.LAG:
                                    u = step - c.LAG
                                    hsts = []
                                    for ps in range(c.PS1):
                                        hst = hstp.tile([P, c.CT * P], F8, tag="hst")
                                        for r in range(c.NC):
                                            ro = (r * c.PS1 + ps) * P
                                            nc.sync.dma_start(
                                                hst[:, r * c.NOWN:(r + 1) * c.NOWN],
                                                ag_outs[u][ro:ro + P, :])
                                        hsts.append(hst)
                                if step < c.T:
                                    t = step
                                    if t == 0:
                                        xts[0] = p1x.tile([P, c.NOWN], BF, tag="xt", name="xt0")
                                        nc.sync.dma_start(xts[0][:], xT[0])
                                    if t + 1 < c.T:
                                        xts[t + 1] = p1x.tile([P, c.NOWN], BF, tag="xt", name=f"xt{t+1}")
                                        nc.sync.dma_start(xts[t + 1][:], xT[t + 1])
                                    xt = xts.pop(t)
                                    for grp in range(2):
                                        gru1_step(xt, p1w, psg, grp * GW, GW)
                                    # transpose h1_t to node-major (fp8), stage + AllGather
                                    for ps in range(c.PS1):
                                        stg = p1s.tile([P, c.NOWN], F8, tag="stg")
                                        for nt in range(c.NT):
                                            pt = ps1t.tile([P, P], BF, tag="ps_t")
                                            nc.tensor.transpose(
                                                pt[:],
                                                h1[:, ps * c.NOWN + nt * P:ps * c.NOWN + (nt + 1) * P],
                                                ident_sb[:])
                                            nc.vector.tensor_copy(stg[:, nt * P:(nt + 1) * P], pt[:])
                                        nc.sync.dma_start(ag_in[t, ps * P:(ps + 1) * P, :], stg[:])
                                    nc.gpsimd.collective_compute(
                                        "AllGather", mybir.AluOpType.bypass, replica_groups=rg,
                                        ins=[ag_in[t].opt()], outs=[ag_outs[t].opt()])
                                if step >= c.LAG:
                                    u = step - c.LAG
                                    a1t = a1p.tile([P, c.PS1 * c.NOWN], F8, tag="a1t")
                                    for q in range(4):
                                        mt3 = mtq[q][:].rearrange("p (ct f) -> p ct f", f=QT)
                                        for ps in range(c.PS1):
                                            hst3 = hsts[ps][:].rearrange("p (ct f) -> p ct f", f=P)
                                            pa = aps.tile([P, QT], F32, tag="pa")
                                            for cp in range(c.CT // 2):
                                                nc.tensor.matmul(
                                                    pa[:], hst3[:, 2 * cp:2 * cp + 2, :],
                                                    mt3[:, 2 * cp:2 * cp + 2, :],
                                                    start=(cp == 0), stop=(cp == c.CT // 2 - 1),
                                                    perf_mode=DR)
                                            nc.scalar.activation(
                                                a1t[:, ps * c.NOWN + q * QT:ps * c.NOWN + (q + 1) * QT],
                                                pa[:], Relu, bias=bias_sb[:, CB1 + ps:CB1 + ps + 1])
                                    a1t_3 = a1t[:].rearrange("p (c n) -> p c n", n=c.NOWN)
                                    for grp in range(2):
                                        gru2_step(a1t_3, p3w, psg, grp * GW, GW)

                    # ---- tail: h2 . G matvec + AllReduce + softmax ----
                    HC = 64
                    NCK = c.NOWN // HC
                    with tc.tile_pool(name="p4", bufs=1) as p4, \
                         tc.tile_pool(name="p4w", bufs=NCK) as p4w, \
                         tc.tile_pool(name="ps4", bufs=1, space="PSUM") as ps4:
                        plog = ps4.tile([1, c.OUT], F32, tag="plog")
                        # prefetch all of G during the pipeline drain
                        lws = []
                        for ck in range(NCK):
                            n0 = ck * HC
                            nt, no = n0 // P, n0 % P
                            lw = p4w.tile([P, HC * c.OUT], F8, tag="lw", name=f"lw{ck}")
                            nc.sync.dma_start(
                                lw[:].rearrange("p (n j) -> p n j", j=c.OUT),
                                linW4[nt, :, no:no + HC, :])
                            lws.append(lw)
                        n_mm = 0
                        for ck in range(NCK):
                            n0 = ck * HC
                            lw = lws[ck]
                            for ni in range(HC):
                                nc.tensor.matmul(plog[:], h2[:, n0 + ni:n0 + ni + 1],
                                                 lw[:, ni * c.OUT:(ni + 1) * c.OUT],
                                                 start=(n_mm == 0),
                                                 stop=(n_mm == c.NOWN - 1))
                                n_mm += 1
                        lpart = p4.tile([1, c.OUT], F32)
                        nc.scalar.activation(lpart[:], plog[:], Copy, scale=1.0 / GSCALE)
                        nc.sync.dma_start(ar_in[:], lpart[:])
                        nc.gpsimd.collective_compute(
                            "AllReduce", mybir.AluOpType.add, replica_groups=rg,
                            ins=[ar_in.opt()], outs=[ar_out.opt()])
                        lg = p4.tile([1, c.OUT], F32)
                        nc.sync.dma_start(lg[:], ar_out[:])
                        nc.vector.tensor_add(lg[:], lg[:], linc_sb[:])
                        mx = p4.tile([1, 1], F32)
                        nc.vector.tensor_reduce(mx[:], lg[:], mybir.AxisListType.X,
                                                mybir.AluOpType.max, negate=True)
                        ex = p4.tile([1, c.OUT], F32)
                        nc.scalar.activation(ex[:], lg[:], Exp, bias=mx[:, 0:1])
                        sm = p4.tile([1, 1], F32)
                        nc.vector.tensor_reduce(sm[:], ex[:], mybir.AxisListType.X,
                                                mybir.AluOpType.add)
                        rcp = p4.tile([1, 1], F32)
                        nc.vector.reciprocal(rcp[:], sm[:])
                        res = p4.tile([1, c.OUT], F32)
                        nc.vector.tensor_scalar_mul(res[:], ex[:], rcp[:, 0:1])
                        nc.sync.dma_start(out[:], res[:])

    nc.compile()
    return nc


def host_prep(cfg: Cfg, x, edge_index, W_ih1, W_hh1, b_ih1, b_hh1, bias1,
              W_ih2, W_hh2, b_ih2, b_hh2, bias2, lin_W, lin_b):
    """Shard + preprocess FULL inputs into per-core in_maps."""
    import scipy.sparse as sp
    c = cfg
    x = np.asarray(x, np.float32)
    edge_index = np.asarray(edge_index)
    # normalized adjacency with self loops: M[dst, src]
    row, col = edge_index[0], edge_index[1]
    loops = np.arange(c.N, dtype=row.dtype)
    row = np.concatenate([row, loops])
    col = np.concatenate([col, loops])
    deg = np.zeros(c.N, np.float32)
    np.add.at(deg, col, 1.0)
    dis = np.where(deg > 0, deg ** -0.5, 0.0).astype(np.float32)
    norm = (dis[row] * dis[col]).astype(np.float32)
    M = np.zeros((c.NPAD, c.NPAD), np.float32)
    np.add.at(M, (col, row), norm)

    xp = np.zeros((c.T, c.NPAD, c.DIN), np.float32)
    xp[:, :c.N, :] = x

    W_ih1 = np.asarray(W_ih1, np.float32); W_hh1 = np.asarray(W_hh1, np.float32)
    W_ih2 = np.asarray(W_ih2, np.float32); W_hh2 = np.asarray(W_hh2, np.float32)
    b_ih1 = np.asarray(b_ih1, np.float32); b_hh1 = np.asarray(b_hh1, np.float32)
    b_ih2 = np.asarray(b_ih2, np.float32); b_hh2 = np.asarray(b_hh2, np.float32)
    bias1 = np.asarray(bias1, np.float32); bias2 = np.asarray(bias2, np.float32)
    lin_W = np.asarray(lin_W, np.float32); lin_b = np.asarray(lin_b, np.float32)

    def dr_layout(WT, G):
        # [2*128, G] -> [128, 2, G] -> [128, 2*G] fp8 (DoubleRow stationary)
        return np.ascontiguousarray(
            WT.reshape(2, P, G).transpose(1, 0, 2).reshape(P, 2 * G)).astype(E4M3)

    bias_cols = np.zeros((P, NBIAS), np.float32)
    bias_cols[:, B1RZ:B1RZ + 4] = (b_ih1 + b_hh1)[:2 * c.H1].reshape(4, P).T
    bias_cols[:, B1HN:B1HN + 2] = b_hh1[2 * c.H1:].reshape(2, P).T
    bias_cols[:, B1IN:B1IN + 2] = b_ih1[2 * c.H1:].reshape(2, P).T
    bias_cols[:, B2RZ:B2RZ + 2] = (b_ih2 + b_hh2)[:2 * c.H2].reshape(2, P).T
    bias_cols[:, B2HN] = b_hh2[2 * c.H2:]
    bias_cols[:, B2IN] = b_ih2[2 * c.H2:]
    bias_cols[:, CB1:CB1 + 2] = bias1.reshape(2, P).T

    # fold the final Linear through aggregation 2:
    #   logits = sum_{s,f} G[o,s,f] h2[s,f] + linc,  G = einsum('odf,ds->osf')
    lw = np.zeros((c.OUT, c.NPAD, c.H2), np.float32)
    lw[:, :c.N, :] = lin_W.reshape(c.OUT, c.N, c.H2)
    St = sp.coo_matrix((norm, (row, col)), shape=(c.NPAD, c.NPAD)).tocsr()  # M^T
    LWd = np.ascontiguousarray(lw.transpose(1, 0, 2).reshape(c.NPAD, c.OUT * c.H2))
    G = St @ LWd  # (NPAD, OUT*H2)
    G = np.clip(G * GSCALE, -240.0, 240.0)
    linc_v = (np.einsum('odf,f->o', lw, bias2) + lin_b).reshape(1, c.OUT).astype(np.float32)

    common = dict(
        wih1T=W_ih1.T.astype(BF16),
        whh1d=dr_layout(W_hh1.T.astype(np.float32), c.G1),
        wih2d=dr_layout(W_ih2.T.astype(np.float32), c.G2),
        whh2T=W_hh2.T.astype(BF16),
        biases=bias_cols,
        linc=linc_v,
        ident=np.eye(P, dtype=BF16),
    )

    in_maps = []
    for k in range(c.NC):
        sl = slice(k * c.NOWN, (k + 1) * c.NOWN)
        m = dict(common)
        m["xT"] = np.ascontiguousarray(xp[:, sl, :].transpose(0, 2, 1)).astype(BF16)
        # M^T slice for this core's dest nodes, pre-tiled: (4, P, CT, NOWN//4)
        mk = M[sl, :].T.reshape(c.CT, P, 4, c.NOWN // 4)
        m["mTt"] = np.ascontiguousarray(mk.transpose(2, 1, 0, 3)).astype(E4M3)
        # G slice: (NOWN, OUT*H2) -> (NT, H2, P, OUT)
        gk = G[sl].reshape(c.NT, P, c.OUT, c.H2)
        m["linW4"] = np.ascontiguousarray(gk.transpose(0, 3, 1, 2)).astype(E4M3)
        in_maps.append(m)
    return in_maps


_CACHE = {}


def _get_built(key, cfg):
    if key not in _CACHE:
        _CACHE[key] = build(cfg)
    return _CACHE[key]


def run(cfg: Cfg, inputs, trace=False):
    _install_ntff_hook()
    nc = _get_built(("cfg", cfg.T, cfg.N), cfg)
    in_maps = host_prep(cfg, **inputs)
    res = run_bass_kernel_spmd(nc, in_maps, core_ids=list(range(cfg.NC)), trace=trace)
    return res


def kernel(**inputs) -> np.ndarray:
    cfg = Cfg()
    res = run(cfg, inputs)
    return np.asarray(res.results[0]["out"], np.float32)
